# revision 1
# baseline (speedup 1.0000x reference)
"""Distributed kNN-classifier kernel for Trainium2 (8 NeuronCores).

Strategy (classic distributed kNN, column-sharded):
  - distances [2048, 100000] f32 are sharded along the prototype (column)
    dim: core c gets columns [c*12500, (c+1)*12500).
  - On device (per core, per 128-row tile): grouped min over groups of 25
    columns (one streaming TensorReduce pass, negated output), then three
    rounds of max8 / max_index / match_replace select the 24 groups with
    the smallest group-minima per row.  Only the group ids [2048, 24] u16
    leave the device.
  - Host: the 8*24 candidate groups per row (4800 candidate columns) are
    gathered from the input, reduced to the exact global top-16 by
    (value, column-index) lexicographic order (bit-exact vs jax.lax.top_k
    tie semantics), labels looked up, and the mode-with-smallest-label
    vote computed exactly as the reference does.

Exactness argument: an element of per-core rank r lives in a group whose
min is among the r smallest element values, hence among the r
lexicographically-smallest group minima; selecting 24 groups therefore
covers every element of per-core rank <= 24, which covers the global
top-16 plus any realistic tie multiplicity at the boundary.
"""

import os
import sys

import numpy as np

sys.path.insert(0, "/opt/trn_rl_repo")

import concourse.bass as bass
import concourse.mybir as mybir
from concourse.tile import TileContext
from concourse.bass_utils import run_bass_kernel_spmd

R = 2048          # rows (batch)
N = 100000        # prototypes (columns)
NC = 8            # cores
S = N // NC       # 12500 columns per core
G = 25            # group size
NG = S // G       # 500 groups per row per core
NSEL = 24         # groups selected per row per core (3 rounds of max8)
K = 16
NUM_CLASSES = 100
P = 128           # partitions
NT = R // P       # 16 row-tiles

_CACHE = {}


NBUF = 3  # SBUF slots for the big data tiles


def build_nc():
    """Raw-Bass SPMD program (no Tile): walrus can encode at most one sync
    wait on a DMA instruction, so all waits are standalone sequencer
    instructions with explicit semaphores.

    SP engine: streams the 16 big tile loads (slot ring of NBUF), then one
    output DMA.  DVE: per tile, grouped-min reduce + 3x(max8/max_index/
    match_replace) rounds.  red_sem releases a slot as soon as its reduce
    (the only reader of the big tile) finished; dve_sem counts finished
    tiles for the final output DMA.
    """
    nc = bass.Bass()
    din = nc.declare_dram_parameter("d", [R, S], mybir.dt.float32, isOutput=False)
    gout = nc.declare_dram_parameter("gidx", [R, NSEL], mybir.dt.uint16, isOutput=True)

    with (
        nc.sbuf_tensor([P, NBUF * S], mybir.dt.float32) as tiles,
        nc.sbuf_tensor([P, NG], mybir.dt.float32) as gneg,
        nc.sbuf_tensor([P, 8], mybir.dt.float32) as m8,
        nc.sbuf_tensor([P, NT * NSEL], mybir.dt.uint16) as gidx_all,
        nc.semaphore("dma_sem") as dma_sem,
        nc.semaphore("red_sem") as red_sem,
        nc.semaphore("dve_sem") as dve_sem,
        nc.Block() as block,
    ):

        @block.sync
        def _(sync):
            for t in range(NT):
                if t >= NBUF:
                    # slot's previous tile fully consumed by its reduce
                    sync.wait_ge(red_sem, t - NBUF + 1)
                s = t % NBUF
                sync.dma_start(
                    out=tiles[:, s * S : (s + 1) * S],
                    in_=din[t * P : (t + 1) * P, :],
                ).then_inc(dma_sem, 16)
            sync.wait_ge(dve_sem, NT)
            sync.dma_start(
                out=gout.rearrange("(t p) s -> p t s", p=P),
                in_=gidx_all[:].rearrange("p (t s) -> p t s", s=NSEL),
            ).then_inc(dma_sem, 16)
            sync.wait_ge(dma_sem, 16 * (NT + 1))

        @block.vector
        def _(vector):
            for t in range(NT):
                vector.wait_ge(dma_sem, 16 * (t + 1))
                s = t % NBUF
                # gneg[p, g] = -min over group of d = max over group of -d
                nc.vector.tensor_reduce(
                    out=gneg[:],
                    in_=tiles[:, s * S : (s + 1) * S].rearrange(
                        "p (g e) -> p g e", e=G
                    ),
                    axis=mybir.AxisListType.X,
                    op=mybir.AluOpType.min,
                    negate=True,
                ).then_inc(red_sem, 1)
                # DVE writes retire ~8 pipe stages after the next
                # instruction's reads issue: every write->read pair needs an
                # explicit drain (read->write pairs are safe).
                nc.vector.drain()
                for r in range(NSEL // 8):
                    nc.vector.max(out=m8[:], in_=gneg[:])
                    nc.vector.drain()
                    nc.vector.max_index(
                        out=gidx_all[:, t * NSEL + r * 8 : t * NSEL + (r + 1) * 8],
                        in_max=m8[:],
                        in_values=gneg[:],
                    )
                    if r < NSEL // 8 - 1:
                        nc.vector.match_replace(
                            out=gneg[:],
                            in_to_replace=m8[:],
                            in_values=gneg[:],
                            imm_value=-3.0e38,
                        )
                        nc.vector.drain()
                nc.vector.drain().then_inc(dve_sem, 1)

    return nc


def _sortable_u32(vals_f32):
    b = vals_f32.view(np.uint32)
    return np.where(b & 0x80000000, ~b, b | np.uint32(0x80000000)).astype(np.uint32)


def host_finish(g_idx_all, d, labels):
    """g_idx_all: [NC, R, NSEL] selected group ids. Returns winning labels [R]."""
    cols = (
        g_idx_all.transpose(1, 0, 2)[:, :, :, None].astype(np.int32) * G
        + np.arange(G, dtype=np.int32)[None, None, None, :]
        + (np.arange(NC, dtype=np.int32) * S)[None, :, None, None]
    ).reshape(R, -1)
    vals = np.take_along_axis(d, cols, axis=1)
    key = (_sortable_u32(vals).astype(np.uint64) << np.uint64(17)) | cols.astype(
        np.uint64
    )
    key = np.partition(key, K - 1, axis=1)[:, :K]
    key.sort(axis=1)
    top_cols = (key[:, :K] & np.uint64(0x1FFFF)).astype(np.int64)
    gathered = labels[top_cols]  # [R, K]
    eq = gathered[:, :, None] == gathered[:, None, :]
    counts = eq.sum(axis=-1)
    score = counts.astype(np.int64) * (NUM_CLASSES + 1) - gathered
    idx = np.argmax(score, axis=1)
    return np.take_along_axis(gathered, idx[:, None], axis=1)[:, 0]


def run_device(d, trace=False):
    if "nc" not in _CACHE:
        _CACHE["nc"] = build_nc()
    nc = _CACHE["nc"]
    in_maps = [
        {"d": np.ascontiguousarray(d[:, c * S : (c + 1) * S])} for c in range(NC)
    ]
    res = run_bass_kernel_spmd(nc, in_maps, list(range(NC)), trace=trace)
    g_idx_all = np.stack(
        [np.asarray(res.results[c]["gidx"]).astype(np.int64) for c in range(NC)]
    )
    return g_idx_all, res


def kernel(distances, labels):
    d = np.ascontiguousarray(np.asarray(distances, dtype=np.float32))
    lab = np.asarray(labels)
    g_idx_all, _ = run_device(d)
    out = host_finish(g_idx_all, d, lab.astype(np.int64))
    return out.astype(lab.dtype)



# revision 2
# speedup vs baseline: 1.1005x; 1.1005x over previous
"""Distributed kNN-classifier kernel for Trainium2 (8 NeuronCores).

Strategy (classic distributed kNN, column-sharded):
  - distances [2048, 100000] f32 are sharded along the prototype (column)
    dim: core c gets columns [c*12500, (c+1)*12500).
  - On device (per core, per 128-row tile): grouped min over groups of 25
    columns (one streaming TensorReduce pass, negated output), then two
    rounds of max8 / max_index / match_replace select the 16 groups with
    the smallest group-minima per row.  Only the group ids [2048, 16] u16
    leave the device.
  - Host: the 8*16 candidate groups per row (3200 candidate columns) are
    gathered from the input, reduced to the exact global top-16 by
    (value, column-index) lexicographic order (bit-exact vs jax.lax.top_k
    tie semantics), labels looked up, and the mode-with-smallest-label
    vote computed exactly as the reference does.

Exactness argument (16 groups suffice, even with exact f32 ties): let e
be an element of global rank <= 16 (by (value, col) order).  Within its
core, every group ranked before group(e) by (min-value, group-id) order
contains an element that precedes e in (value, col) order: strictly
smaller minima contribute strictly smaller elements, and equal minima
with smaller group id contribute an equal-valued element at a smaller
column (group ids are column-ordered).  Hence group(e)'s rank is <= 16.
The device select (max8 + max_index first-occurrence-dedup + match
_replace) realizes exactly this (min-value, group-id) order.

Pipeline (raw Bass, per core): 15 full row-tiles of [128, 12500] plus a
last tile processed as 4 column-chunks of 3125 so only ~7us of DVE work
remains after the final HBM byte lands.  The [2048, 16] result is
written as two DMAs (tiles 0-14 overlap the last tile's compute).
"""

import sys

import numpy as np

sys.path.insert(0, "/opt/trn_rl_repo")

import concourse.bass as bass
import concourse.mybir as mybir
from concourse.bass_utils import run_bass_kernel_spmd

R = 2048          # rows (batch)
N = 100000        # prototypes (columns)
NC = 8            # cores
S = N // NC       # 12500 columns per core
G = 25            # group size
NG = S // G       # 500 groups per row per core
NSEL = 16         # groups selected per row per core (2 rounds of max8)
K = 16
NUM_CLASSES = 100
P = 128           # partitions
NT = R // P       # 16 row-tiles
NCHUNK = 4        # the last row-tile is processed in 4 column-chunks
CS = S // NCHUNK  # 3125 columns per chunk
CG = NG // NCHUNK # 125 groups per chunk

_CACHE = {}


NBUF = 3  # SBUF slots for the big data tiles


def build_nc():
    """Raw-Bass SPMD program (no Tile): walrus can encode at most one sync
    wait on a DMA instruction, so all waits are standalone sequencer
    instructions with explicit semaphores.

    SP engine: streams the tile loads (slot ring of NBUF; tile 15 split
    into NCHUNK column-chunk DMAs), then the two output DMAs.  DVE: per
    tile, grouped-min reduce + 2x(max8/max_index[/match_replace]) rounds.
    red_sem releases a slot as soon as its reduce (the only reader of the
    big tile) finished; dve_sem counts finished tiles for the output DMAs.
    """
    nc = bass.Bass()
    din = nc.declare_dram_parameter("d", [R, S], mybir.dt.float32, isOutput=False)
    # [P, NT*NSEL]: per-partition contiguous so the output DMA is one
    # large-descriptor transfer; host transposes (t p) -> rows.
    gout = nc.declare_dram_parameter(
        "gidx", [P, NT * NSEL], mybir.dt.uint16, isOutput=True
    )

    n_in_dmas = (NT - 1) + NCHUNK

    with (
        nc.sbuf_tensor([P, NBUF * S], mybir.dt.float32) as tiles,
        nc.sbuf_tensor([P, NG], mybir.dt.float32) as gneg,
        nc.sbuf_tensor([P, 8], mybir.dt.float32) as m8,
        nc.sbuf_tensor([P, NT * NSEL], mybir.dt.uint16) as gidx_all,
        nc.semaphore("dma_sem") as dma_sem,
        nc.semaphore("red_sem") as red_sem,
        nc.semaphore("dve_sem") as dve_sem,
        nc.Block() as block,
    ):

        @block.sync
        def _(sync):
            for t in range(NT - 1):
                if t >= NBUF:
                    # slot's previous tile fully consumed by its reduce
                    sync.wait_ge(red_sem, t - NBUF + 1)
                s = t % NBUF
                sync.dma_start(
                    out=tiles[:, s * S : (s + 1) * S],
                    in_=din[t * P : (t + 1) * P, :],
                ).then_inc(dma_sem, 16)
            # last tile, 4 column-chunks into slot (NT-1) % NBUF
            sync.wait_ge(red_sem, NT - NBUF)
            s = (NT - 1) % NBUF
            for c in range(NCHUNK):
                sync.dma_start(
                    out=tiles[:, s * S + c * CS : s * S + (c + 1) * CS],
                    in_=din[(NT - 1) * P : NT * P, c * CS : (c + 1) * CS],
                ).then_inc(dma_sem, 16)
            # output DMAs: tiles 0..14 early (overlaps last tile compute)
            sync.wait_ge(dve_sem, NT - 1)
            sync.dma_start(
                out=gout[:, : (NT - 1) * NSEL],
                in_=gidx_all[:, : (NT - 1) * NSEL],
            ).then_inc(dma_sem, 16)
            sync.wait_ge(dve_sem, NT)
            sync.dma_start(
                out=gout[:, (NT - 1) * NSEL :],
                in_=gidx_all[:, (NT - 1) * NSEL :],
            ).then_inc(dma_sem, 16)
            sync.wait_ge(dma_sem, 16 * (n_in_dmas + 2))

        def select(vector, t):
            """Two max8 rounds over gneg -> gidx_all[:, t*NSEL:(t+1)*NSEL].
            Caller guarantees gneg writes are drained."""
            for r in range(NSEL // 8):
                nc.vector.max(out=m8[:], in_=gneg[:])
                nc.vector.drain()
                nc.vector.max_index(
                    out=gidx_all[:, t * NSEL + r * 8 : t * NSEL + (r + 1) * 8],
                    in_max=m8[:],
                    in_values=gneg[:],
                )
                if r < NSEL // 8 - 1:
                    nc.vector.match_replace(
                        out=gneg[:],
                        in_to_replace=m8[:],
                        in_values=gneg[:],
                        imm_value=-3.0e38,
                    )
                    nc.vector.drain()
            nc.vector.drain().then_inc(dve_sem, 1)

        @block.vector
        def _(vector):
            for t in range(NT - 1):
                vector.wait_ge(dma_sem, 16 * (t + 1))
                s = t % NBUF
                # gneg[p, g] = -min over group of d = max over group of -d
                nc.vector.tensor_reduce(
                    out=gneg[:],
                    in_=tiles[:, s * S : (s + 1) * S].rearrange(
                        "p (g e) -> p g e", e=G
                    ),
                    axis=mybir.AxisListType.X,
                    op=mybir.AluOpType.min,
                    negate=True,
                ).then_inc(red_sem, 1)
                # DVE writes retire ~8 pipe stages after the next
                # instruction's reads issue: every write->read pair needs an
                # explicit drain (read->write pairs are safe).
                nc.vector.drain()
                select(vector, t)
            # last tile: reduce per chunk as each chunk lands
            s = (NT - 1) % NBUF
            for c in range(NCHUNK):
                vector.wait_ge(dma_sem, 16 * (NT + c))
                nc.vector.tensor_reduce(
                    out=gneg[:, c * CG : (c + 1) * CG],
                    in_=tiles[:, s * S + c * CS : s * S + (c + 1) * CS].rearrange(
                        "p (g e) -> p g e", e=G
                    ),
                    axis=mybir.AxisListType.X,
                    op=mybir.AluOpType.min,
                    negate=True,
                )
            nc.vector.drain()
            select(vector, NT - 1)

    return nc


def _sortable_u32(vals_f32):
    b = vals_f32.view(np.uint32)
    return np.where(b & 0x80000000, ~b, b | np.uint32(0x80000000)).astype(np.uint32)


def host_finish(g_idx_all, d, labels):
    """g_idx_all: [NC, R, NSEL] selected group ids. Returns winning labels [R]."""
    cols = (
        g_idx_all.transpose(1, 0, 2)[:, :, :, None].astype(np.int32) * G
        + np.arange(G, dtype=np.int32)[None, None, None, :]
        + (np.arange(NC, dtype=np.int32) * S)[None, :, None, None]
    ).reshape(R, -1)
    vals = np.take_along_axis(d, cols, axis=1)
    key = (_sortable_u32(vals).astype(np.uint64) << np.uint64(17)) | cols.astype(
        np.uint64
    )
    key = np.partition(key, K - 1, axis=1)[:, :K]
    key.sort(axis=1)
    top_cols = (key[:, :K] & np.uint64(0x1FFFF)).astype(np.int64)
    gathered = labels[top_cols]  # [R, K]
    eq = gathered[:, :, None] == gathered[:, None, :]
    counts = eq.sum(axis=-1)
    score = counts.astype(np.int64) * (NUM_CLASSES + 1) - gathered
    idx = np.argmax(score, axis=1)
    return np.take_along_axis(gathered, idx[:, None], axis=1)[:, 0]


def run_device(d, trace=False):
    if "nc" not in _CACHE:
        _CACHE["nc"] = build_nc()
    nc = _CACHE["nc"]
    in_maps = [
        {"d": np.ascontiguousarray(d[:, c * S : (c + 1) * S])} for c in range(NC)
    ]
    res = run_bass_kernel_spmd(nc, in_maps, list(range(NC)), trace=trace)
    # gout is [P, NT*NSEL] with row r = t*P + p at gout[p, t*NSEL:(t+1)*NSEL]
    g_idx_all = np.stack(
        [
            np.asarray(res.results[c]["gidx"])
            .reshape(P, NT, NSEL)
            .transpose(1, 0, 2)
            .reshape(R, NSEL)
            .astype(np.int64)
            for c in range(NC)
        ]
    )
    return g_idx_all, res


def kernel(distances, labels):
    d = np.ascontiguousarray(np.asarray(distances, dtype=np.float32))
    lab = np.asarray(labels)
    g_idx_all, _ = run_device(d)
    out = host_finish(g_idx_all, d, lab.astype(np.int64))
    return out.astype(lab.dtype)
